# revision 31
# baseline (speedup 1.0000x reference)
"""BitLinear (ternary weight quantization + linear) on 8 TRN2 NeuronCores.

y = x @ w_eff.T with w_eff = clip(round(w/scale), -1, 1) * scale,
scale = clamp(mean |w| per row, 1e-5).

Sharding: column-parallel - weight rows (out_features) split 8 ways; each
core computes y[:, shard] for the full x; host concatenates.

Strategy: the matmul runs in fp8 (e4m3) with MatmulPerfMode.DoubleRow,
which streams 2 MACs/PE/cycle on TRN2 - 2x the fp32r/bf16 rate. Each
DoubleRow instruction contracts 256 k-values (two 128-planes).

 - x is quantized to e4m3 on the host and shipped pre-transposed
   (k-major) so the device does no transposes for x. To control the
   e4m3 quantization noise (full-K single pass measures 1.98e-2
   absmax-rel, too close to the 2e-2 gate) a residual correction
   x_lo = e4m3(x - e4m3(x)) is shipped for the first LO_KP*256 of the
   2048 k's and accumulated into the same PSUM group (extra k-planes
   against the same weights). LO_KP=4 measures 1.44e-2.
 - w is quantized ON DEVICE with exactly the baseline's recipe
   (ACT-engine |w| row-sum -> scale, DVE compares) because that
   bit-matches the reference's own device-computed scale; a host-side
   numpy mean differs by 1 ulp on some rows and flips one ternary
   value, costing 1.35e-2 of absmax error on its own. The ternary
   {-1,0,1} result is cast to fp8 (exact) and PE-transposed to k-major.
 - Output is computed as y^T (out-features in partitions) so the
   per-row scale is applied as a per-partition activation scale during
   PSUM eviction, in bf16 (host upcasts and transposes).

Per-core dataflow:
  W phase (8 chunks of 128 rows): DMA fp32 w chunk, scale+ternarize,
  PE-transpose (fp32r) in 4-block PSUM batches, evict as fp8 into the
  resident k-major weight tile.
  MM phase: for each m-group (4 chunks of 512 rows) stream
  o-tile x k-pair x m-chunk DoubleRow matmuls: stationary
  w8T [128,2,128] (reloaded per 4 moving matmuls - LDWEIGHTS hides
  under the 216ns matmul stream), moving x8T [128,2,512], PSUM
  accumulation over 8+LO_KP k-pairs, ACT eviction with per-partition
  scale to bf16, DMA out y^T strips.
"""

import numpy as np
import ml_dtypes

import concourse.bass as bass
import concourse.mybir as mybir
import concourse.tile as tile
from concourse import bacc
from concourse.bass_utils import run_bass_kernel_spmd
from concourse.masks import make_identity

F32 = mybir.dt.float32
F32R = mybir.dt.float32r
F8 = mybir.dt.float8e4
BF16 = mybir.dt.bfloat16
E4NP = ml_dtypes.float8_e4m3fn

# Problem shape (hardcoded per contract)
B, S, D_IN, D_OUT = 4, 2048, 2048, 8192
NCORES = 8
R = B * S                 # 8192 rows of x
O = D_OUT // NCORES       # 1024 out features per core
OT = O // 128             # 8 o-tiles (weight chunks)
KP_HI = D_IN // 256       # 8 hi k-pairs
# 4 lo k-pairs = residual correction for the first 1024 of 2048 k's.
# Measured absmax-rel 1.44e-2 / l2-rel 1.67e-2 vs the reference -- safely
# under the 2e-2 gate under either metric. (LO_KP=2 would be 55us faster
# but its l2-rel is 2.04e-2, a fail if the gate metric is l2.)
LO_KP = 4                 # lo-residual k-pairs (256 k each) appended
KP = KP_HI + LO_KP
MG = 4                    # m-groups
MC = 4                    # m-chunks of 512 per group (4*4*512 = 8192)
DR = mybir.MatmulPerfMode.DoubleRow


def _build():
    nc = bacc.Bacc(None, target_bir_lowering=False)

    x_d = nc.dram_tensor("x", [MG, MC, 128, KP * 2 * 512], F8,
                         kind="ExternalInput")
    w_d = nc.dram_tensor("w", [O, D_IN], F32, kind="ExternalInput")
    y_d = nc.dram_tensor("y", [O, R], BF16, kind="ExternalOutput")

    with tile.TileContext(nc) as tc:
        with (
            tc.tile_pool(name="const", bufs=1) as const,
            tc.tile_pool(name="wres", bufs=1) as wres,
            tc.tile_pool(name="ws", bufs=1) as ws,
            tc.tile_pool(name="xs", bufs=2) as xs,
            tc.tile_pool(name="ys", bufs=2) as ysp,
            tc.tile_pool(name="pst", bufs=2, space="PSUM") as pst,
            tc.tile_pool(name="psm", bufs=1, space="PSUM") as psm,
        ):
            ident_f = const.tile([128, 128], F32)
            make_identity(nc, ident_f[:])
            ident = const.tile([128, 128], F32R)
            nc.vector.tensor_copy(ident[:], ident_f[:])

            # resident k-major fp8 weights: [p, kp, i, o] , k = 256kp+128i+p
            w8t = wres.tile([128, KP_HI, 2, O], F8)
            # per-o-tile scale vectors (partition-aligned with y^T psum)
            scales = [wres.tile([128, 1], F32, name=f"scale_{a}")
                      for a in range(OT)]

            def w_chunk(a):
                """Quantize + transpose weight rows a*128..(a+1)*128.

                Scale recipe matches the fp32r baseline bit-for-bit (ACT
                Abs+accum row-sum, DVE mult/max) - proven to reproduce the
                reference's device-computed scale exactly.
                """
                w_in = ws.tile([128, D_IN], F32, tag="w_in", bufs=3,
                               name=f"w_in_{a}")
                nc.sync.dma_start(w_in[:], w_d[a * 128:(a + 1) * 128, :])

                absdump = ws.tile([128, D_IN], F32, tag="w_tmp", bufs=2,
                                  name=f"absdump_{a}")
                ssum = ws.tile([128, 1], F32, tag="w_sum", name=f"ssum_{a}")
                nc.scalar.activation(
                    absdump[:], w_in[:],
                    mybir.ActivationFunctionType.Abs,
                    accum_out=ssum[:],
                )
                scale = scales[a]
                nc.vector.tensor_scalar(
                    out=scale[:], in0=ssum[:], scalar1=1.0 / D_IN,
                    scalar2=1e-5, op0=mybir.AluOpType.mult,
                    op1=mybir.AluOpType.max,
                )
                hpos = ws.tile([128, 1], F32, tag="w_hpos", name=f"hp_{a}")
                nc.vector.tensor_scalar_mul(hpos[:], scale[:], 0.5)

                # ternary in {-1, 0, 1} as sign(w) * (|w| > 0.5*scale);
                # identical decision to (w>h)-(w<-h) including fp32 compare
                # semantics, one fewer DVE pass (Sign runs on ACT).
                sgn = ws.tile([128, D_IN], F32, tag="w_sgn", bufs=2,
                              name=f"sgn_{a}")
                nc.scalar.activation(
                    sgn[:], w_in[:], mybir.ActivationFunctionType.Sign
                )
                ge = ws.tile([128, D_IN], F32, tag="w_pos", name=f"ge_{a}")
                nc.vector.tensor_scalar(
                    out=ge[:], in0=absdump[:], scalar1=hpos[:], scalar2=1.0,
                    op0=mybir.AluOpType.is_gt, op1=mybir.AluOpType.mult,
                )
                wpm = ws.tile([128, D_IN], F32R, tag="w_pm", name=f"wpm_{a}")
                nc.vector.tensor_mul(wpm[:], sgn[:], ge[:])

                # PE-transpose [128o, 2048k] -> k-major, 4 blocks per bank
                for jb in range(4):
                    pt = pst.tile([128, 512], F32, tag="wtps", bufs=1,
                                  name=f"wpt_{a}_{jb}")
                    for t in range(4):
                        j = jb * 4 + t
                        nc.tensor.transpose(
                            pt[:, t * 128:(t + 1) * 128].bitcast(F32R),
                            wpm[:, j * 128:(j + 1) * 128],
                            ident[:],
                        )
                    # dst free dims (kp:2, i:2, o:128) match pt's 512
                    dst = w8t[:, 2 * jb:2 * jb + 2, :, a * 128:(a + 1) * 128]
                    nc.scalar.copy(
                        dst.rearrange("p a b c -> p (a b) c"),
                        pt[:].rearrange("p (t c) -> p t c", t=4),
                    )

            def mm_group(g, a, xg):
                """o-tile a against m-group g: accumulate, scale, store."""
                # mc0/mc1 double-buffered (8 banks total with the 2
                # transpose banks): consecutive groups overlap their
                # eviction waits on the first two banks
                accs = [
                    psm.tile([128, 512], F32, tag=f"mm{mc}",
                             bufs=(2 if mc < 3 else 1),
                             name=f"acc_{g}_{a}_{mc}")
                    for mc in range(MC)
                ]
                for kp in range(KP):
                    wk = kp if kp < KP_HI else kp - KP_HI
                    for mc in range(MC):
                        nc.tensor.matmul(
                            accs[mc][:],
                            w8t[:, wk, :, a * 128:(a + 1) * 128],
                            xg[mc][:, kp, :, :],
                            start=(kp == 0),
                            stop=(kp == KP - 1),
                            perf_mode=DR,
                        )
                yst = ysp.tile([128, MC * 512], BF16, tag="yst",
                               name=f"yst_{g}_{a}")
                final = (g == MG - 1) and (a == OT - 1)
                for mc in range(MC):
                    # evictions split ACT/DVE: each engine's in-order queue
                    # otherwise serializes psum-reuse behind W-quant work
                    if mc < 2:
                        nc.scalar.activation(
                            yst[:, mc * 512:(mc + 1) * 512],
                            accs[mc][:],
                            mybir.ActivationFunctionType.Copy,
                            scale=scales[a][:],
                        )
                    else:
                        nc.vector.tensor_scalar_mul(
                            yst[:, mc * 512:(mc + 1) * 512],
                            accs[mc][:],
                            scales[a][:],
                        )
                    if final:
                        # per-mc strips so the tail DMA overlaps the last
                        # evictions instead of waiting for all four
                        lo = g * 2048 + mc * 512
                        nc.sync.dma_start(
                            y_d[a * 128:(a + 1) * 128, lo:lo + 512],
                            yst[:, mc * 512:(mc + 1) * 512],
                        )
                if not final:
                    nc.sync.dma_start(
                        y_d[a * 128:(a + 1) * 128, g * 2048:(g + 1) * 2048],
                        yst[:],
                    )

            def x_slice(g, mc):
                """One contiguous [128, KP*2*512] slice (12KB lines)."""
                t = xs.tile([128, KP, 2, 512], F8, tag=f"xg{mc}",
                            name=f"xg_{g}_{mc}")
                nc.sync.dma_start(
                    t[:].rearrange("p a b c -> p (a b c)"), x_d[g, mc]
                )
                return t

            # DMA/emission order tuned for the head: w0 lands first (its
            # quant chain ~14us is the critical path to the first matmul),
            # x slices for m-group 0 right behind it, then chunks 1-3;
            # chunks 4-7 interleave with the first m-group's o-tiles
            # (each ~10us of PE stream). The next m-group's x DMA is
            # issued one group early to hide its ~19us under the stream.
            w_chunk(0)
            xg0 = [x_slice(0, mc) for mc in range(MC)]
            for a in range(1, 4):
                w_chunk(a)
            xgs = {0: xg0}
            for a in range(OT):
                mm_group(0, a, xg0)
                if a + 4 < OT:
                    w_chunk(a + 4)
                if a == 4:
                    # prefetch after the w-chunk DMAs have queue priority
                    xgs[1] = [x_slice(1, mc) for mc in range(MC)]
            for g in range(1, MG):
                for a in range(OT):
                    mm_group(g, a, xgs[g])
                    if a == 0 and g + 1 < MG:
                        xgs[g + 1] = [x_slice(g + 1, mc) for mc in range(MC)]

    nc.compile()
    return nc


_NC_CACHE = None


def _get_nc():
    global _NC_CACHE
    if _NC_CACHE is None:
        _NC_CACHE = _build()
    return _NC_CACHE


def _pack_x(x: np.ndarray) -> np.ndarray:
    """e4m3 hi + partial lo residual, k-major packed [128, KP, 2, R]."""
    xf = np.ascontiguousarray(x.reshape(R, D_IN), dtype=np.float32)
    x8 = xf.astype(E4NP)
    if LO_KP:
        resid = xf[:, : LO_KP * 256] - x8[:, : LO_KP * 256].astype(np.float32)
        xcat = np.concatenate([x8, resid.astype(E4NP)], axis=1)
    else:
        xcat = x8
    xT = np.ascontiguousarray(xcat.T)  # [KP*256, R]
    # [MG, MC, p, kp, i, m] so each (g, mc) slice is one contiguous DMA
    return np.ascontiguousarray(
        xT.reshape(KP, 2, 128, MG, MC, 512).transpose(3, 4, 2, 0, 1, 5)
    ).reshape(MG, MC, 128, KP * 2 * 512)


def kernel(x: np.ndarray, weight: np.ndarray, _trace: bool = False):
    assert x.shape == (B, S, D_IN) and weight.shape == (D_OUT, D_IN)
    x_st = _pack_x(x)
    in_maps = [
        {
            "x": x_st,
            "w": np.ascontiguousarray(
                weight[c * O:(c + 1) * O], dtype=np.float32
            ),
        }
        for c in range(NCORES)
    ]
    nc = _get_nc()
    res = run_bass_kernel_spmd(
        nc, in_maps, core_ids=list(range(NCORES)), trace=_trace
    )
    y = np.concatenate(
        [
            np.asarray(res.results[c]["y"]).astype(np.float32).T
            for c in range(NCORES)
        ],
        axis=1,
    )
    out = np.ascontiguousarray(y.reshape(B, S, D_OUT))
    if _trace:
        return out, res
    return out
